# revision 33
# baseline (speedup 1.0000x reference)
"""Trainium2 Bass kernel for nn_AttentionScore (sparse local attention scores).

Reference computation (B=4, C=64, N=16384, S=16):
    tmp   = xyz[:, :, :, None] - neighbor_xyz            # [B,3,N,S]
    pos   = concat([tmp, ||tmp||], axis=1)               # [B,4,N,S]
    k     = Wk @ (neighbor_points + Wpos @ pos + bpos)   # [B,C,N,S]
    attn  = softmax_s((points*scale) . k)                # [B,N,S]

Softmax over s is shift-invariant, so every term constant in s drops out:
    attn[m,s] ~ sum_c qW[c,m]*np[c,m,s] + sum_j qp[j,m]*tmp[j,m,s] + qp3[m]*||tmp||
with qW = (scale*Wk)^T @ points, qp = Wpos^T @ qW (bpos and the xyz.qp dot cancel).

Sharding: N split contiguously across 8 cores (no communication needed).
m = b*2048 + n_local in [0, 8192) per core.

v3: all big HBM streams staged bf16 (halves DMA traffic vs f32); DVE ops
arranged for the 2x_1p packed mode (bf16 + innermost unit-stride pairs —
broadcasts made packable by pair-duplication); phase 1 fully front-loaded;
qp accumulated into one persistent PSUM bank via per-chunk column-shifted
selector weights (one PSUM->SBUF copy instead of 16); selector matrix HS
host-built; weight builds + square on GpSimd; softmax skips the max
subtraction (scores bounded ~+-4, f32 exp is safe).

Layouts per core (M = 8192 rows, d = (m//512)%2, t = m//1024, mm = m%512):
  NP  [128=(d,c), (t8, mm512, s)] bf16
  NX  [128=m//64, (mi, j, s)] bf16
  XYZ2[128=m//64, (mi, j, 2dup)] bf16
  P   [64=c, m] bf16
  HS  [128=(d,c), (k16, col32)] bf16 selector: col d*16+k of window k is 1
  OUT [128=m//64, (mi, s)] f32

Main loop: 16 half-supertiles (t, h), each 256 mm x 2 d-groups = 512 m:
DVE multiplies np by pair-duplicated qW (bf16 2x), TensorE reduces the 64
c-partitions per d-group with selector matmuls (8 x 512 cols into one
[32, 512] PSUM tile, row d*16+k = 32-mm chunk k), ScalarE copies PSUM ->
SBUF bf16 and triggers the partition-scatter SBUF->SBUF DMA into the
softmax layout attn1[p=m//64, (m%64)*16+s].
"""

import os
import sys

sys.path.insert(0, "/opt/trn_rl_repo")

import numpy as np
import ml_dtypes

import concourse.bass as bass
import concourse.bacc as bacc
import concourse.tile as tile
from concourse import mybir
from concourse.bass_utils import run_bass_kernel_spmd

F32 = mybir.dt.float32
BF16 = mybir.dt.bfloat16
AF = mybir.ActivationFunctionType
AX = mybir.AxisListType
OP = mybir.AluOpType

BF = ml_dtypes.bfloat16

B, C, N, S = 4, 64, 16384, 16
NCORES = 8
NL = N // NCORES            # 2048 points per core
M = B * NL                  # 8192 (b, n) rows per core
MB = 256                    # mm per half-supertile per d-group
NT = 16                     # half-supertiles, each covering 512 m
CH = 512                    # phase-1 q chunk (one d-block)
NC1 = M // CH               # 16 chunks
SCALE = float(C) ** -0.5


def _body(tc):
    nc = tc.nc
    dma = nc.sync.dma_start

    NP = nc.dram_tensor("NP", [128, M * S // 2], BF16, kind="ExternalInput").ap()
    NX = nc.dram_tensor("NX", [128, 64 * 3 * S], BF16, kind="ExternalInput").ap()
    XYZ2 = nc.dram_tensor("XYZ2", [128, 64 * 3 * 2], BF16, kind="ExternalInput").ap()
    P = nc.dram_tensor("P", [C, M], BF16, kind="ExternalInput").ap()
    HS = nc.dram_tensor("HS", [128, 16 * 32], BF16, kind="ExternalInput").ap()
    WALL = nc.dram_tensor("WALL", [C, 132], F32, kind="ExternalInput").ap()
    OUT = nc.dram_tensor("OUT", [128, (M // 128) * S], BF16, kind="ExternalOutput").ap()

    with (
        tc.tile_pool(name="const", bufs=1) as cp,
        tc.tile_pool(name="small", bufs=1) as sp,
        tc.tile_pool(name="w3072", bufs=1) as p3072,
        tc.tile_pool(name="w1024", bufs=4) as p1024,
        # main loop pools
        tc.tile_pool(name="npt", bufs=4) as npp,
        tc.tile_pool(name="prod", bufs=3) as prp,
        tc.tile_pool(name="nptx", bufs=1) as nppx,
        tc.tile_pool(name="prodx", bufs=1) as prpx,
        tc.tile_pool(name="sc", bufs=4) as scp,
        tc.tile_pool(name="psq", bufs=2, space="PSUM") as psq,
        tc.tile_pool(name="psw", bufs=1, space="PSUM") as psw,
        tc.tile_pool(name="psm", bufs=4, space="PSUM") as psm,
    ):
        # ---- weights + P ride the Sync queue AHEAD of the NP stream so the
        # phase-1 chain starts as early as possible ----
        wall = cp.tile([C, 132], F32)
        dma(wall[:], WALL)
        wk = wall[:, 0:64]
        wkt = wall[:, 64:128]
        wp = wall[:, 128:132]
        hs = cp.tile([128, 16 * 32], BF16)
        dma(hs[:], HS)
        pta = cp.tile([C, M // 2], BF16)
        dma(pta[:], P[:, 0:M // 2])
        ptb = cp.tile([C, M // 2], BF16)
        dma(ptb[:], P[:, M // 2:M])
        # phase-2 inputs on the ACT queue (needed later)
        xyz2 = cp.tile([128, 64 * 3 * 2], BF16)
        nc.scalar.dma_start(xyz2[:], XYZ2)
        nxt = cp.tile([128, 64 * 3 * S], BF16)
        nc.scalar.dma_start(nxt[:], NX)

        # ---- tiny weight prep (DVE for the casts, GpSimd for the builds) ----
        wks = sp.tile([C, C], BF16)
        nc.vector.tensor_scalar_mul(wks[:], wk, SCALE)
        wkts = sp.tile([C, C], BF16)
        nc.vector.tensor_scalar_mul(wkts[:], wkt, SCALE)
        wpb = sp.tile([C, 4], BF16)
        nc.vector.tensor_copy(wpb[:], wp)

        # Wkp[c, j] = sum_c' (scale*Wk)[c, c'] Wpos[c', j]
        pwkp = psw.tile([C, 4], F32)
        nc.tensor.matmul(pwkp[:], lhsT=wkts[:], rhs=wpb[:], start=True, stop=True)
        wkp = sp.tile([C, 4], BF16)
        nc.scalar.copy(wkp[:], pwkp[:])

        # Fused phase-1 weights, one per d-group: [64, 128] with cols
        # d*64..+64 = scale*Wk (-> qW on out rows d*64+c) and cols
        # (1-d)*64..+4 = Wkp (-> qp on out rows (1-d)*64+j). One matmul per
        # chunk produces both.
        whs = []
        for d in range(2):
            wh = sp.tile([C, 128], BF16, name=f"wh{d}", tag=f"wh{d}")
            nc.gpsimd.memset(wh[:], 0.0)
            nc.gpsimd.tensor_copy(wh[:, d * 64:d * 64 + 64], wks[:])
            nc.gpsimd.tensor_copy(wh[:, (1 - d) * 64:(1 - d) * 64 + 4], wkp[:])
            whs.append(wh)

        qw2 = cp.tile([128, M], BF16)            # [(d,c), (t, mm512, 2dup)]
        qpt = cp.tile([128, 4 * 64], F32)        # [m//64, (j, mi)]
        qps = cp.tile([4, M], F32)               # [j, m] staging
        attn1 = cp.tile([128, 64 * S], BF16)
        attn2 = cp.tile([128, 64 * S], BF16)

        # ---- phase 1, fully front-loaded ----
        for cc in range(NC1):
            d, t = cc % 2, cc // 2
            pch = pta if cc < 8 else ptb
            rhs = pch[:, (cc % 8) * CH:(cc % 8 + 1) * CH]

            pq = psq.tile([128, CH], F32)
            nc.tensor.matmul(pq[:], lhsT=whs[d][:], rhs=rhs, start=True, stop=True)
            # qW rows -> qw2, duplicated into adjacent pairs (bf16)
            nc.scalar.copy(
                qw2[d * 64:d * 64 + 64, t * 2 * CH:(t + 1) * 2 * CH].rearrange(
                    "p (mm two) -> p mm two", two=2
                ),
                pq[d * 64:d * 64 + 64, :]
                .rearrange("p (mm one) -> p mm one", one=1)
                .broadcast_to((64, CH, 2)),
            )
            # qp rows -> wide staging, in q-block column order (the chunk's
            # two h-halves land at q-blocks 16t+8h+4d..+4) so the qpt
            # scatter below lands rows in the same q order as attn1
            qpsv = qps[:].rearrange(
                "p (t h d k2 w) -> p t h d k2 w", t=8, h=2, d=2, k2=4, w=64
            )[:, t, :, d, :, :]
            nc.scalar.copy(
                qpsv,
                pq[(1 - d) * 64:(1 - d) * 64 + 4, :].rearrange(
                    "p (h k2 w) -> p h k2 w", h=2, k2=4, w=64
                ),
            )

        for j in range(4):
            # qpt[m//64, (j, m%64)] <- qps[j, m]
            nc.scalar.dma_start(qpt[:, j * 64:(j + 1) * 64], qps[j:j + 1, :])

        # ---- phase 2a: positional distances (no qp dependency) ----
        def phase2a():
            nx4 = nxt[:].rearrange(
                "p (mi j s2 two) -> p mi j s2 two", mi=64, j=3, s2=S // 2, two=2
            )
            xyzb = (
                xyz2[:]
                .rearrange("p (mi j one two) -> p mi j one two", mi=64, j=3, one=1, two=2)
                .broadcast_to((128, 64, 3, S // 2, 2))
            )
            tmp = p3072.tile([128, 64 * 3 * S], BF16, tag="big")
            tmp4 = tmp[:].rearrange(
                "p (mi j s2 two) -> p mi j s2 two", mi=64, j=3, s2=S // 2, two=2
            )
            nc.vector.tensor_sub(tmp4, xyzb, nx4)

            sq = p3072.tile([128, 64 * 3 * S], BF16, tag="big2")
            nc.gpsimd.tensor_mul(sq[:], tmp[:], tmp[:])

            def jsl(tl, j):
                return tl[:].rearrange(
                    "p (mi j s2 two) -> p mi j s2 two", mi=64, j=3, s2=S // 2, two=2
                )[:, :, j, :, :]

            na = p1024.tile([128, 64 * S], BF16, tag="w1k")
            na3 = na[:].rearrange("p (mi s2 two) -> p mi s2 two", s2=S // 2, two=2)
            nc.vector.tensor_add(na3, jsl(sq, 0), jsl(sq, 1))
            norm2 = p1024.tile([128, 64 * S], BF16, tag="w1k")
            n23 = norm2[:].rearrange("p (mi s2 two) -> p mi s2 two", s2=S // 2, two=2)
            nc.vector.tensor_add(n23, na3, jsl(sq, 2))
            norm = p1024.tile([128, 64 * S], BF16, tag="w1k")
            nc.scalar.sqrt(norm[:], norm2[:])
            # dummy exp right after the sqrt: swaps the ACT table to Exp
            # mid-stream so the tail's real exp pays no table reload
            junk = sp.tile([C, 4], F32)
            nc.scalar.activation(junk[:], wkp[:], AF.Exp)
            ph2_state["tmp"] = tmp
            ph2_state["norm"] = norm
            ph2_state["jsl"] = jsl

        # ---- phase 2b: qp-weighted sum (needs qpt complete) ----
        def phase2b():
            tmp = ph2_state["tmp"]
            norm = ph2_state["norm"]
            jsl = ph2_state["jsl"]

            qpt2 = sp.tile([128, 4 * 64 * 2], BF16)
            nc.vector.tensor_copy(
                qpt2[:].rearrange("p (j mi two) -> p j mi two", j=4, two=2),
                qpt[:]
                .rearrange("p (j mi one) -> p j mi one", j=4, one=1)
                .broadcast_to((128, 4, 64, 2)),
            )

            def qsl(j):
                return (
                    qpt2[:, j * 128:(j + 1) * 128]
                    .rearrange("p (mi one two) -> p mi one two", one=1, two=2)
                    .broadcast_to((128, 64, S // 2, 2))
                )

            ua = p1024.tile([128, 64 * S], BF16, tag="w1k")
            ua3 = ua[:].rearrange("p (mi s2 two) -> p mi s2 two", s2=S // 2, two=2)
            nc.vector.tensor_mul(ua3, jsl(tmp, 0), qsl(0))
            ub = p1024.tile([128, 64 * S], BF16, tag="w1k")
            ub3 = ub[:].rearrange("p (mi s2 two) -> p mi s2 two", s2=S // 2, two=2)
            nc.vector.tensor_mul(ub3, jsl(tmp, 1), qsl(1))
            nc.vector.tensor_add(ua3, ua3, ub3)
            nc.vector.tensor_mul(ub3, jsl(tmp, 2), qsl(2))
            nc.vector.tensor_add(ua3, ua3, ub3)

            a23 = ub3
            nc.vector.tensor_mul(
                a23,
                norm[:].rearrange("p (mi s2 two) -> p mi s2 two", s2=S // 2, two=2),
                qsl(3),
            )
            nc.vector.tensor_add(
                attn2[:].rearrange("p (mi s2 two) -> p mi s2 two", s2=S // 2, two=2),
                ua3,
                a23,
            )

        ph2_state = {}

        # ---- main loop: half-supertiles; kl0/nkl select a chunk sub-range
        # so the final tile can be split small to shorten the tail ----
        def supertile(T, kl0=0, nkl=8):
            t, h = T // 2, T % 2
            mm0 = kl0 * 32
            full = kl0 == 0 and nkl == 8
            npt = (npp if full else nppx).tile(
                [128, nkl * 32 * S], BF16, name="npt", tag=f"npt{nkl}"
            )
            base = (t * 2 + h) * MB * S + mm0 * S
            dma(npt[:], NP[:, base:base + nkl * 32 * S])

            prod = (prp if full else prpx).tile(
                [128, nkl * 32 * S], BF16, name="prod", tag=f"prod{nkl}"
            )
            qwb = (
                qw2[:, t * 1024 + h * 512 + mm0 * 2:t * 1024 + h * 512 + (mm0 + nkl * 32) * 2]
                .rearrange("p (mm one two) -> p mm one two", one=1, two=2)
                .broadcast_to((128, nkl * 32, S // 2, 2))
            )
            nc.vector.tensor_mul(
                prod[:].rearrange("p (mm s2 two) -> p mm s2 two", s2=S // 2, two=2),
                npt[:].rearrange("p (mm s2 two) -> p mm s2 two", s2=S // 2, two=2),
                qwb,
            )

            ps = psm.tile([32, 512], F32)
            for i in range(nkl):
                kl = kl0 + i
                k = 8 * h + kl
                nc.tensor.matmul(
                    ps[:],
                    lhsT=hs[:, k * 32:(k + 1) * 32],
                    rhs=prod[:, i * 512:(i + 1) * 512],
                    start=(i == 0),
                    stop=(i == nkl - 1),
                )
            # PSUM row h*16 + d*8 + kl = chunk kl of group d (other rows
            # zero); one contiguous copy + per-d scatter into attn1 row
            # q = 8T + 4d + kl//2, col (kl%2)*512 + (mm%32)*16 + s (host
            # unscrambles q -> m).
            sc = scp.tile([32, 512], BF16)
            nc.scalar.copy(sc[:], ps[:])
            if kl0 == 0 and nkl == 8:
                nc.gpsimd.dma_start(
                    attn1[T * 8:(T + 1) * 8, :].rearrange("p (k1 f) -> p k1 f", k1=2),
                    sc[h * 16:h * 16 + 16, :],
                )
            else:
                for d in range(2):
                    p0 = T * 8 + d * 4 + kl0 // 2
                    nc.gpsimd.dma_start(
                        attn1[p0:p0 + nkl // 2, :].rearrange(
                            "p (k1 f) -> p k1 f", k1=2
                        ),
                        sc[h * 16 + d * 8 + kl0:h * 16 + d * 8 + kl0 + nkl, :],
                    )

        for T in range(NT):
            if T == NT - 1:
                # split the last tile 6+2 chunks so the final DMA->softmax
                # chain after the stream ends is short
                supertile(T, 0, 6)
                supertile(T, 6, 2)
            else:
                supertile(T)
            if T == 1:
                phase2a()
            if T == 11:
                phase2b()

        # ---- softmax over s (no max subtraction: |attn| <= ~6) ----
        attn = p1024.tile([128, 64 * S], F32, tag="w1kf")
        nc.vector.tensor_add(attn[:], attn1[:], attn2[:])

        e = p1024.tile([128, 64 * S], F32, tag="w1kf")
        nc.scalar.activation(e[:], attn[:], AF.Exp)

        se = sp.tile([128, 64], F32)
        nc.vector.reduce_sum(se[:], e[:].rearrange("p (mi s) -> p mi s", mi=64), axis=AX.X)
        rse = sp.tile([128, 64], F32)
        nc.vector.reciprocal(rse[:], se[:])

        o = p1024.tile([128, 64 * S], BF16, tag="w1kb")
        rb = rse[:].rearrange("p (mi one) -> p mi one", one=1).broadcast_to((128, 64, S))
        nc.vector.tensor_mul(
            o[:].rearrange("p (mi s) -> p mi s", mi=64),
            e[:].rearrange("p (mi s) -> p mi s", mi=64),
            rb,
        )
        dma(OUT, o[:])


_NC_CACHE = None


def build_nc():
    global _NC_CACHE
    if _NC_CACHE is None:
        nc = bacc.Bacc(trn_type="TRN2", target_bir_lowering=False, debug=False)
        with tile.TileContext(nc) as tc:
            _body(tc)
        nc.compile()
        _NC_CACHE = nc
    return _NC_CACHE


def make_hs():
    # window k (k = 8h+kl): col h*16 + d*8 + kl is 1 on the d-group rows
    hs = np.zeros((128, 16, 32), dtype=BF)
    for k in range(16):
        h, kl = k // 8, k % 8
        hs[0:64, k, h * 16 + kl] = 1
        hs[64:128, k, h * 16 + 8 + kl] = 1
    return np.ascontiguousarray(hs.reshape(128, 512))


_HS = None
# q-row <-> standard m-block permutation: swap the d (bit 3) and h (bit 2)
# fields of the 64-m block index (involution)
_QPERM = (np.arange(128) & ~0b1100) | ((np.arange(128) & 8) >> 1) | ((np.arange(128) & 4) << 1)


def make_in_maps(xyz, neighbor_xyz, points, neighbor_points, Wk, Wpos, bpos):
    """Slice + relayout + bf16-cast full inputs into the 8 per-core maps."""
    global _HS
    if _HS is None:
        _HS = make_hs()
    xyz = np.asarray(xyz, dtype=np.float32)
    neighbor_xyz = np.asarray(neighbor_xyz, dtype=np.float32)
    points = np.asarray(points, dtype=np.float32)
    neighbor_points = np.asarray(neighbor_points, dtype=np.float32)
    Wk = np.asarray(Wk, dtype=np.float32)
    wall = np.ascontiguousarray(
        np.concatenate([Wk, Wk.T, np.asarray(Wpos, dtype=np.float32)], axis=1)
    )

    in_maps = []
    for i in range(NCORES):
        nsl = slice(i * NL, (i + 1) * NL)
        # np: [B,C,nl,S] -> [c, m, s] -> [(d,c), (t, mm, s)] bf16
        npc = (
            neighbor_points[:, :, nsl, :]
            .transpose(1, 0, 2, 3)
            .reshape(C, M, S)
            .astype(BF)
        )
        npc = (
            npc.reshape(C, 8, 2, 512, S)
            .transpose(2, 0, 1, 3, 4)
            .reshape(128, M * S // 2)
        )
        # nx: [B,3,nl,S] -> [m, j, s] -> [128, (mi, j, s)] bf16, rows in the
        # scatter q-order (64-m block q holds m-block with d/h bits swapped)
        nxc = (
            neighbor_xyz[:, :, nsl, :]
            .transpose(1, 0, 2, 3)
            .reshape(3, M, S)
            .transpose(1, 0, 2)
            .reshape(128, 64 * 3 * S)
            .astype(BF)
        )[_QPERM]
        # xyz: [B,3,nl] -> [m, j] -> duplicated pairs [128, (mi, j, 2)] bf16
        xc = xyz[:, :, nsl].transpose(1, 0, 2).reshape(3, M).T.astype(BF)
        xc2 = np.repeat(xc, 2, axis=1).reshape(128, 64 * 3 * 2)[_QPERM]
        # points: [B,C,nl] -> [c, m] bf16
        pc = points[:, :, nsl].transpose(1, 0, 2).reshape(C, M).astype(BF)
        in_maps.append(
            {
                "NP": np.ascontiguousarray(npc),
                "NX": np.ascontiguousarray(nxc),
                "XYZ2": np.ascontiguousarray(xc2),
                "P": np.ascontiguousarray(pc),
                "HS": _HS,
                "WALL": wall,
            }
        )
    return in_maps


_M0S = None


def assemble_output(results):
    """Per-core OUT [128, 1024] bf16 (scatter row order) -> full [B, N, S] f32.

    Row q = 8T + 4d + k2 (T = 2t+h) covers m = 1024t + 512d + 256h + 64*k2 +
    [0, 64), with the 64 m's in flat col order (k1, ml, s) = ((m%64)//32,
    m%32, s)."""
    global _M0S
    if _M0S is None:
        q = np.arange(128)
        T, r = q // 8, q % 8
        t, h, d, k2 = T // 2, T % 2, r // 4, r % 4
        m0 = 1024 * t + 512 * d + 256 * h + 64 * k2
        _M0S = (m0[:, None] + np.arange(64)[None, :]).ravel()
    out = np.empty((B, N, S), dtype=np.float32)
    for i in range(NCORES):
        oc = np.asarray(results[i]["OUT"]).astype(np.float32).reshape(128 * 64, S)
        flat = np.empty((M, S), dtype=np.float32)
        flat[_M0S] = oc
        out[:, i * NL:(i + 1) * NL, :] = flat.reshape(B, NL, S)
    return out


def run_cores(in_maps, trace=False, trace_kwargs=None):
    nc = build_nc()
    return run_bass_kernel_spmd(
        nc,
        in_maps,
        core_ids=list(range(NCORES)),
        trace=trace,
        **(trace_kwargs or {}),
    )


def kernel(xyz, neighbor_xyz, points, neighbor_points, Wk, Wpos, bpos):
    in_maps = make_in_maps(
        xyz, neighbor_xyz, points, neighbor_points, Wk, Wpos, bpos
    )
    res = run_cores(in_maps, trace=False)
    return assemble_output(res.results)


# revision 34
# speedup vs baseline: 1.1347x; 1.1347x over previous
"""Trainium2 Bass kernel for nn_AttentionScore (sparse local attention scores).

Reference computation (B=4, C=64, N=16384, S=16):
    tmp   = xyz[:, :, :, None] - neighbor_xyz            # [B,3,N,S]
    pos   = concat([tmp, ||tmp||], axis=1)               # [B,4,N,S]
    k     = Wk @ (neighbor_points + Wpos @ pos + bpos)   # [B,C,N,S]
    attn  = softmax_s((points*scale) . k)                # [B,N,S]

Softmax over s is shift-invariant, so every term constant in s drops out:
    attn[m,s] ~ sum_c qW[c,m]*np[c,m,s] + sum_j qp[j,m]*tmp[j,m,s] + qp3[m]*||tmp||
with qW = (scale*Wk)^T @ points, qp = Wpos^T @ qW (bpos and the xyz.qp dot cancel).

Sharding: N split contiguously across 8 cores (no communication needed).
m = b*2048 + n_local in [0, 8192) per core.

v6: all big HBM streams staged bf16 (halves DMA traffic vs f32, the
binding roofline: ~19.4 MB/core at ~358 GB/s); DVE ops arranged for the
2x_1p packed mode (bf16 + innermost unit-stride pairs — broadcasts made
packable by pair-duplication); phase 1 fully front-loaded with one fused
matmul per chunk producing qW and qp together; selector matrix HS
host-built; weight builds + square on GpSimd; attn1 scatters on SWDGE
(ScalarE-sequencer DMA triggers are ~0.7us each — keep them off ACT);
softmax skips the max subtraction (scores bounded ~+-4, f32 exp is safe)
and the exp ACT table is preloaded right after the sqrt so the tail exp
pays no table reload; the last supertile is split 6+2 chunks to shorten
the post-stream tail; OUT is written bf16 and widened on the host.

Layouts per core (M = 8192 rows, d = (m//512)%2, t = m//1024, mm = m%512):
  NP  [128=(d,c), (t8, mm512, s)] bf16
  NX  [128=m//64, (mi, j, s)] bf16
  XYZ2[128=m//64, (mi, j, 2dup)] bf16
  P   [64=c, m] bf16
  HS  [128=(d,c), (k16, col32)] bf16 selector: col d*16+k of window k is 1
  OUT [128=m//64, (mi, s)] f32

Main loop: 16 half-supertiles (t, h), each 256 mm x 2 d-groups = 512 m:
DVE multiplies np by pair-duplicated qW (bf16 2x), TensorE reduces the 64
c-partitions per d-group with selector matmuls (8 x 512 cols into one
[32, 512] PSUM tile, row d*16+k = 32-mm chunk k), ScalarE copies PSUM ->
SBUF bf16 and triggers the partition-scatter SBUF->SBUF DMA into the
softmax layout attn1[p=m//64, (m%64)*16+s].
"""

import os
import sys

sys.path.insert(0, "/opt/trn_rl_repo")

import numpy as np
import ml_dtypes

import concourse.bass as bass
import concourse.bacc as bacc
import concourse.tile as tile
from concourse import mybir
from concourse.bass_utils import run_bass_kernel_spmd

F32 = mybir.dt.float32
BF16 = mybir.dt.bfloat16
AF = mybir.ActivationFunctionType
AX = mybir.AxisListType
OP = mybir.AluOpType

BF = ml_dtypes.bfloat16

B, C, N, S = 4, 64, 16384, 16
NCORES = 8
NL = N // NCORES            # 2048 points per core
M = B * NL                  # 8192 (b, n) rows per core
MB = 256                    # mm per half-supertile per d-group
NT = 16                     # half-supertiles, each covering 512 m
CH = 512                    # phase-1 q chunk (one d-block)
NC1 = M // CH               # 16 chunks
SCALE = float(C) ** -0.5


def _body(tc):
    nc = tc.nc
    dma = nc.sync.dma_start

    NP = nc.dram_tensor("NP", [128, M * S // 2], BF16, kind="ExternalInput").ap()
    NX = nc.dram_tensor("NX", [128, 64 * 3 * S], BF16, kind="ExternalInput").ap()
    XYZ2 = nc.dram_tensor("XYZ2", [128, 64 * 3 * 2], BF16, kind="ExternalInput").ap()
    P = nc.dram_tensor("P", [C, M], BF16, kind="ExternalInput").ap()
    HS = nc.dram_tensor("HS", [128, 16 * 32], BF16, kind="ExternalInput").ap()
    WALL = nc.dram_tensor("WALL", [C, 132], F32, kind="ExternalInput").ap()
    OUT = nc.dram_tensor("OUT", [128, (M // 128) * S], BF16, kind="ExternalOutput").ap()

    with (
        tc.tile_pool(name="const", bufs=1) as cp,
        tc.tile_pool(name="small", bufs=1) as sp,
        tc.tile_pool(name="w3072", bufs=1) as p3072,
        tc.tile_pool(name="w1024", bufs=4) as p1024,
        # main loop pools
        tc.tile_pool(name="npt", bufs=4) as npp,
        tc.tile_pool(name="prod", bufs=3) as prp,
        tc.tile_pool(name="nptx", bufs=1) as nppx,
        tc.tile_pool(name="prodx", bufs=1) as prpx,
        tc.tile_pool(name="sc", bufs=4) as scp,
        tc.tile_pool(name="psq", bufs=2, space="PSUM") as psq,
        tc.tile_pool(name="psw", bufs=1, space="PSUM") as psw,
        tc.tile_pool(name="psm", bufs=4, space="PSUM") as psm,
    ):
        # ---- weights + P ride the Sync queue AHEAD of the NP stream so the
        # phase-1 chain starts as early as possible ----
        wall = cp.tile([C, 132], F32)
        dma(wall[:], WALL)
        wk = wall[:, 0:64]
        wkt = wall[:, 64:128]
        wp = wall[:, 128:132]
        hs = cp.tile([128, 16 * 32], BF16)
        dma(hs[:], HS)
        pta = cp.tile([C, M // 2], BF16)
        dma(pta[:], P[:, 0:M // 2])
        ptb = cp.tile([C, M // 2], BF16)
        dma(ptb[:], P[:, M // 2:M])
        # phase-2 inputs on the ACT queue (needed later)
        xyz2 = cp.tile([128, 64 * 3 * 2], BF16)
        nc.scalar.dma_start(xyz2[:], XYZ2)
        nxt = cp.tile([128, 64 * 3 * S], BF16)
        nc.scalar.dma_start(nxt[:], NX)

        # ---- tiny weight prep (DVE for the casts, GpSimd for the builds) ----
        wks = sp.tile([C, C], BF16)
        nc.vector.tensor_scalar_mul(wks[:], wk, SCALE)
        wkts = sp.tile([C, C], BF16)
        nc.vector.tensor_scalar_mul(wkts[:], wkt, SCALE)
        wpb = sp.tile([C, 4], BF16)
        nc.vector.tensor_copy(wpb[:], wp)

        # Wkp[c, j] = sum_c' (scale*Wk)[c, c'] Wpos[c', j]
        pwkp = psw.tile([C, 4], F32)
        nc.tensor.matmul(pwkp[:], lhsT=wkts[:], rhs=wpb[:], start=True, stop=True)
        wkp = sp.tile([C, 4], BF16)
        nc.scalar.copy(wkp[:], pwkp[:])

        # Fused phase-1 weights, one per d-group: [64, 128] with cols
        # d*64..+64 = scale*Wk (-> qW on out rows d*64+c) and cols
        # (1-d)*64..+4 = Wkp (-> qp on out rows (1-d)*64+j). One matmul per
        # chunk produces both.
        whs = []
        for d in range(2):
            wh = sp.tile([C, 128], BF16, name=f"wh{d}", tag=f"wh{d}")
            nc.gpsimd.memset(wh[:], 0.0)
            nc.gpsimd.tensor_copy(wh[:, d * 64:d * 64 + 64], wks[:])
            nc.gpsimd.tensor_copy(wh[:, (1 - d) * 64:(1 - d) * 64 + 4], wkp[:])
            whs.append(wh)

        qw2 = cp.tile([128, M], BF16)            # [(d,c), (t, mm512, 2dup)]
        qpt = cp.tile([128, 4 * 64], F32)        # [m//64, (j, mi)]
        qps = cp.tile([4, M], F32)               # [j, m] staging
        attn1 = cp.tile([128, 64 * S], BF16)
        attn2 = cp.tile([128, 64 * S], BF16)

        # ---- phase 1, fully front-loaded ----
        for cc in range(NC1):
            d, t = cc % 2, cc // 2
            pch = pta if cc < 8 else ptb
            rhs = pch[:, (cc % 8) * CH:(cc % 8 + 1) * CH]

            pq = psq.tile([128, CH], F32)
            nc.tensor.matmul(pq[:], lhsT=whs[d][:], rhs=rhs, start=True, stop=True)
            # qW rows -> qw2, duplicated into adjacent pairs (bf16)
            nc.scalar.copy(
                qw2[d * 64:d * 64 + 64, t * 2 * CH:(t + 1) * 2 * CH].rearrange(
                    "p (mm two) -> p mm two", two=2
                ),
                pq[d * 64:d * 64 + 64, :]
                .rearrange("p (mm one) -> p mm one", one=1)
                .broadcast_to((64, CH, 2)),
            )
            # qp rows -> wide staging, in q-block column order (the chunk's
            # two h-halves land at q-blocks 16t+8h+4d..+4) so the qpt
            # scatter below lands rows in the same q order as attn1
            qpsv = qps[:].rearrange(
                "p (t h d k2 w) -> p t h d k2 w", t=8, h=2, d=2, k2=4, w=64
            )[:, t, :, d, :, :]
            nc.scalar.copy(
                qpsv,
                pq[(1 - d) * 64:(1 - d) * 64 + 4, :].rearrange(
                    "p (h k2 w) -> p h k2 w", h=2, k2=4, w=64
                ),
            )

        for j in range(4):
            # qpt[m//64, (j, m%64)] <- qps[j, m]
            nc.scalar.dma_start(qpt[:, j * 64:(j + 1) * 64], qps[j:j + 1, :])

        # ---- phase 2a: positional distances (no qp dependency) ----
        def phase2a():
            nx4 = nxt[:].rearrange(
                "p (mi j s2 two) -> p mi j s2 two", mi=64, j=3, s2=S // 2, two=2
            )
            xyzb = (
                xyz2[:]
                .rearrange("p (mi j one two) -> p mi j one two", mi=64, j=3, one=1, two=2)
                .broadcast_to((128, 64, 3, S // 2, 2))
            )
            tmp = p3072.tile([128, 64 * 3 * S], BF16, tag="big")
            tmp4 = tmp[:].rearrange(
                "p (mi j s2 two) -> p mi j s2 two", mi=64, j=3, s2=S // 2, two=2
            )
            nc.vector.tensor_sub(tmp4, xyzb, nx4)

            sq = p3072.tile([128, 64 * 3 * S], BF16, tag="big2")
            nc.gpsimd.tensor_mul(sq[:], tmp[:], tmp[:])

            def jsl(tl, j):
                return tl[:].rearrange(
                    "p (mi j s2 two) -> p mi j s2 two", mi=64, j=3, s2=S // 2, two=2
                )[:, :, j, :, :]

            na = p1024.tile([128, 64 * S], BF16, tag="w1k")
            na3 = na[:].rearrange("p (mi s2 two) -> p mi s2 two", s2=S // 2, two=2)
            nc.vector.tensor_add(na3, jsl(sq, 0), jsl(sq, 1))
            norm2 = p1024.tile([128, 64 * S], BF16, tag="w1k")
            n23 = norm2[:].rearrange("p (mi s2 two) -> p mi s2 two", s2=S // 2, two=2)
            nc.vector.tensor_add(n23, na3, jsl(sq, 2))
            norm = p1024.tile([128, 64 * S], BF16, tag="w1k")
            nc.scalar.sqrt(norm[:], norm2[:])
            # dummy exp right after the sqrt: swaps the ACT table to Exp
            # mid-stream so the tail's real exp pays no table reload
            junk = sp.tile([C, 4], F32)
            nc.scalar.activation(junk[:], wkp[:], AF.Exp)
            ph2_state["tmp"] = tmp
            ph2_state["norm"] = norm
            ph2_state["jsl"] = jsl

        # ---- phase 2b: qp-weighted sum (needs qpt complete) ----
        def phase2b():
            tmp = ph2_state["tmp"]
            norm = ph2_state["norm"]
            jsl = ph2_state["jsl"]

            qpt2 = sp.tile([128, 4 * 64 * 2], BF16)
            nc.vector.tensor_copy(
                qpt2[:].rearrange("p (j mi two) -> p j mi two", j=4, two=2),
                qpt[:]
                .rearrange("p (j mi one) -> p j mi one", j=4, one=1)
                .broadcast_to((128, 4, 64, 2)),
            )

            def qsl(j):
                return (
                    qpt2[:, j * 128:(j + 1) * 128]
                    .rearrange("p (mi one two) -> p mi one two", one=1, two=2)
                    .broadcast_to((128, 64, S // 2, 2))
                )

            ua = p1024.tile([128, 64 * S], BF16, tag="w1k")
            ua3 = ua[:].rearrange("p (mi s2 two) -> p mi s2 two", s2=S // 2, two=2)
            nc.vector.tensor_mul(ua3, jsl(tmp, 0), qsl(0))
            ub = p1024.tile([128, 64 * S], BF16, tag="w1k")
            ub3 = ub[:].rearrange("p (mi s2 two) -> p mi s2 two", s2=S // 2, two=2)
            nc.vector.tensor_mul(ub3, jsl(tmp, 1), qsl(1))
            nc.vector.tensor_add(ua3, ua3, ub3)
            nc.vector.tensor_mul(ub3, jsl(tmp, 2), qsl(2))
            nc.vector.tensor_add(ua3, ua3, ub3)

            a23 = ub3
            nc.vector.tensor_mul(
                a23,
                norm[:].rearrange("p (mi s2 two) -> p mi s2 two", s2=S // 2, two=2),
                qsl(3),
            )
            nc.vector.tensor_add(
                attn2[:].rearrange("p (mi s2 two) -> p mi s2 two", s2=S // 2, two=2),
                ua3,
                a23,
            )

        ph2_state = {}

        # ---- main loop: half-supertiles; kl0/nkl select a chunk sub-range
        # so the final tile can be split small to shorten the tail ----
        def supertile(T, kl0=0, nkl=8):
            t, h = T // 2, T % 2
            mm0 = kl0 * 32
            full = kl0 == 0 and nkl == 8
            npt = (npp if full else nppx).tile(
                [128, nkl * 32 * S], BF16, name="npt", tag=f"npt{nkl}"
            )
            base = (t * 2 + h) * MB * S + mm0 * S
            dma(npt[:], NP[:, base:base + nkl * 32 * S])

            prod = (prp if full else prpx).tile(
                [128, nkl * 32 * S], BF16, name="prod", tag=f"prod{nkl}"
            )
            qwb = (
                qw2[:, t * 1024 + h * 512 + mm0 * 2:t * 1024 + h * 512 + (mm0 + nkl * 32) * 2]
                .rearrange("p (mm one two) -> p mm one two", one=1, two=2)
                .broadcast_to((128, nkl * 32, S // 2, 2))
            )
            nc.vector.tensor_mul(
                prod[:].rearrange("p (mm s2 two) -> p mm s2 two", s2=S // 2, two=2),
                npt[:].rearrange("p (mm s2 two) -> p mm s2 two", s2=S // 2, two=2),
                qwb,
            )

            ps = psm.tile([32, 512], F32)
            for i in range(nkl):
                kl = kl0 + i
                k = 8 * h + kl
                nc.tensor.matmul(
                    ps[:],
                    lhsT=hs[:, k * 32:(k + 1) * 32],
                    rhs=prod[:, i * 512:(i + 1) * 512],
                    start=(i == 0),
                    stop=(i == nkl - 1),
                )
            # PSUM row h*16 + d*8 + kl = chunk kl of group d (other rows
            # zero); one contiguous copy + per-d scatter into attn1 row
            # q = 8T + 4d + kl//2, col (kl%2)*512 + (mm%32)*16 + s (host
            # unscrambles q -> m).
            sc = scp.tile([32, 512], BF16)
            nc.scalar.copy(sc[:], ps[:])
            if kl0 == 0 and nkl == 8:
                nc.gpsimd.dma_start(
                    attn1[T * 8:(T + 1) * 8, :].rearrange("p (k1 f) -> p k1 f", k1=2),
                    sc[h * 16:h * 16 + 16, :],
                )
            else:
                for d in range(2):
                    p0 = T * 8 + d * 4 + kl0 // 2
                    nc.gpsimd.dma_start(
                        attn1[p0:p0 + nkl // 2, :].rearrange(
                            "p (k1 f) -> p k1 f", k1=2
                        ),
                        sc[h * 16 + d * 8 + kl0:h * 16 + d * 8 + kl0 + nkl, :],
                    )

        for T in range(NT):
            if T == NT - 1:
                # split the last tile 6+2 chunks so the final DMA->softmax
                # chain after the stream ends is short
                supertile(T, 0, 6)
                supertile(T, 6, 2)
            else:
                supertile(T)
            if T == 1:
                phase2a()
            if T == 11:
                phase2b()

        # ---- softmax over s (no max subtraction: |attn| <= ~6) ----
        attn = p1024.tile([128, 64 * S], F32, tag="w1kf")
        nc.vector.tensor_add(attn[:], attn1[:], attn2[:])

        e = p1024.tile([128, 64 * S], F32, tag="w1kf")
        nc.scalar.activation(e[:], attn[:], AF.Exp)

        se = sp.tile([128, 64], F32)
        nc.vector.reduce_sum(se[:], e[:].rearrange("p (mi s) -> p mi s", mi=64), axis=AX.X)
        rse = sp.tile([128, 64], F32)
        nc.vector.reciprocal(rse[:], se[:])

        o = p1024.tile([128, 64 * S], BF16, tag="w1kb")
        rb = rse[:].rearrange("p (mi one) -> p mi one", one=1).broadcast_to((128, 64, S))
        nc.vector.tensor_mul(
            o[:].rearrange("p (mi s) -> p mi s", mi=64),
            e[:].rearrange("p (mi s) -> p mi s", mi=64),
            rb,
        )
        dma(OUT, o[:])


_NC_CACHE = None


def build_nc():
    global _NC_CACHE
    if _NC_CACHE is None:
        nc = bacc.Bacc(trn_type="TRN2", target_bir_lowering=False, debug=False)
        with tile.TileContext(nc) as tc:
            _body(tc)
        nc.compile()
        _NC_CACHE = nc
    return _NC_CACHE


def make_hs():
    # window k (k = 8h+kl): col h*16 + d*8 + kl is 1 on the d-group rows
    hs = np.zeros((128, 16, 32), dtype=BF)
    for k in range(16):
        h, kl = k // 8, k % 8
        hs[0:64, k, h * 16 + kl] = 1
        hs[64:128, k, h * 16 + 8 + kl] = 1
    return np.ascontiguousarray(hs.reshape(128, 512))


_HS = None
# q-row <-> standard m-block permutation: swap the d (bit 3) and h (bit 2)
# fields of the 64-m block index (involution)
_QPERM = (np.arange(128) & ~0b1100) | ((np.arange(128) & 8) >> 1) | ((np.arange(128) & 4) << 1)


def make_in_maps(xyz, neighbor_xyz, points, neighbor_points, Wk, Wpos, bpos):
    """Slice + relayout + bf16-cast full inputs into the 8 per-core maps."""
    global _HS
    if _HS is None:
        _HS = make_hs()
    xyz = np.asarray(xyz, dtype=np.float32)
    neighbor_xyz = np.asarray(neighbor_xyz, dtype=np.float32)
    points = np.asarray(points, dtype=np.float32)
    neighbor_points = np.asarray(neighbor_points, dtype=np.float32)
    Wk = np.asarray(Wk, dtype=np.float32)
    wall = np.ascontiguousarray(
        np.concatenate([Wk, Wk.T, np.asarray(Wpos, dtype=np.float32)], axis=1)
    )

    in_maps = []
    for i in range(NCORES):
        nsl = slice(i * NL, (i + 1) * NL)
        # np: [B,C,nl,S] -> [c, m, s] -> [(d,c), (t, mm, s)] bf16
        npc = (
            neighbor_points[:, :, nsl, :]
            .transpose(1, 0, 2, 3)
            .reshape(C, M, S)
            .astype(BF)
        )
        npc = (
            npc.reshape(C, 8, 2, 512, S)
            .transpose(2, 0, 1, 3, 4)
            .reshape(128, M * S // 2)
        )
        # nx: [B,3,nl,S] -> [m, j, s] -> [128, (mi, j, s)] bf16, rows in the
        # scatter q-order (64-m block q holds m-block with d/h bits swapped)
        nxc = (
            neighbor_xyz[:, :, nsl, :]
            .transpose(1, 0, 2, 3)
            .reshape(3, M, S)
            .transpose(1, 0, 2)
            .reshape(128, 64 * 3 * S)
            .astype(BF)
        )[_QPERM]
        # xyz: [B,3,nl] -> [m, j] -> duplicated pairs [128, (mi, j, 2)] bf16
        xc = xyz[:, :, nsl].transpose(1, 0, 2).reshape(3, M).T.astype(BF)
        xc2 = np.repeat(xc, 2, axis=1).reshape(128, 64 * 3 * 2)[_QPERM]
        # points: [B,C,nl] -> [c, m] bf16
        pc = points[:, :, nsl].transpose(1, 0, 2).reshape(C, M).astype(BF)
        in_maps.append(
            {
                "NP": np.ascontiguousarray(npc),
                "NX": np.ascontiguousarray(nxc),
                "XYZ2": np.ascontiguousarray(xc2),
                "P": np.ascontiguousarray(pc),
                "HS": _HS,
                "WALL": wall,
            }
        )
    return in_maps


_M0S = None


def assemble_output(results):
    """Per-core OUT [128, 1024] bf16 (scatter row order) -> full [B, N, S] f32.

    Row q = 8T + 4d + k2 (T = 2t+h) covers m = 1024t + 512d + 256h + 64*k2 +
    [0, 64), with the 64 m's in flat col order (k1, ml, s) = ((m%64)//32,
    m%32, s)."""
    global _M0S
    if _M0S is None:
        q = np.arange(128)
        T, r = q // 8, q % 8
        t, h, d, k2 = T // 2, T % 2, r // 4, r % 4
        m0 = 1024 * t + 512 * d + 256 * h + 64 * k2
        _M0S = (m0[:, None] + np.arange(64)[None, :]).ravel()
    out = np.empty((B, N, S), dtype=np.float32)
    for i in range(NCORES):
        oc = np.asarray(results[i]["OUT"]).astype(np.float32).reshape(128 * 64, S)
        flat = np.empty((M, S), dtype=np.float32)
        flat[_M0S] = oc
        out[:, i * NL:(i + 1) * NL, :] = flat.reshape(B, NL, S)
    return out


def run_cores(in_maps, trace=False, trace_kwargs=None):
    nc = build_nc()
    return run_bass_kernel_spmd(
        nc,
        in_maps,
        core_ids=list(range(NCORES)),
        trace=trace,
        **(trace_kwargs or {}),
    )


def kernel(xyz, neighbor_xyz, points, neighbor_points, Wk, Wpos, bpos):
    in_maps = make_in_maps(
        xyz, neighbor_xyz, points, neighbor_points, Wk, Wpos, bpos
    )
    res = run_cores(in_maps, trace=False)
    return assemble_output(res.results)
